# revision 16
# baseline (speedup 1.0000x reference)
"""Trainium2 Bass kernel for a prototypical-network classification head.

Math (per task b):
    protos  = one_hot(labels).T @ support / n_shot          # (5, 1024)
    logits  = scale/d * (2*q@protos.T - |q|^2 - |p|^2)       # (75, 5)

Sharding: data-parallel over the 512 tasks across 8 NeuronCores (64 each).

v5 dataflow (zero on-device transposes):
  - Host pre-transposes query to qT (d on partitions, bf16) so the PE
    never transposes anything.  Host label-sorts support, pre-scales it
    by 2/n_shot and casts fp8e4m3 (support only touches the logits
    through class-mean inner products; measured end-to-end rel err
    7.2e-3 vs the 2e-2 gate).
  - protosT built by PE matmuls: stationary = natural-layout support
    chunk (s on partitions, 128 d columns), moving = a tiny
    block-diagonal 0/1 one-hot shared by all tasks.
  - AB^T accumulates in a 3-bank (128, 1200) psum tile per 16-task
    group over 8 d-chunks: stationary = protosT slice (128, 80).
  - AA = |q|^2 per query: ACT/DVE split squares of qT, DVE adds chunk
    pairs, a ones-vector PE matmul reduces partitions into spare psum
    row 96 of the same AB psum tile.  BB likewise from protosT.
  - One K=33 "fold" matmul per psum bank window adds -AA[col] - BB[row]
    (rows 0/32 of the operands are the written lanes; partition bases
    stay 32-aligned for the BIR verifier).  The psum->SBUF copy applies
    scale/d and casts bf16.  Transposed logits ship to DRAM; the host
    extracts each task's (5, 75) diagonal block and transposes it
    (layout-only numpy work).
"""

import numpy as np
from contextlib import ExitStack

import ml_dtypes
import concourse.bass as bass
import concourse.bacc as bacc
import concourse.tile as tile
from concourse import mybir
from concourse import bass_utils

F32 = mybir.dt.float32
BF16 = mybir.dt.bfloat16
FP8 = mybir.dt.float8e4

# Problem shape (hardcoded per the task spec).
B, NQ, NS, D = 512, 75, 25, 1024
NW, NSHOT = 5, 5
NCORES = 8
BPC = B // NCORES          # 64 tasks per core
DC = D // 128              # 8 contraction chunks

TG = 16                    # tasks per AB group
NG = BPC // TG             # 4 groups
GC = TG * NQ               # 1200 q-columns per group
HC = GC // 2               # 600 q-columns per DMA half
PB = 5                     # tasks per protos block (K = 125)
NPB = (BPC + PB - 1) // PB  # 13 blocks (last has 4 tasks)
PTC = BPC * NW             # 320 protosT columns

# psum-bank-aligned column windows of the (128, 1200) group tile, split
# at the DMA-half boundary (qT halves live in separate SBUF regions).
# 4th field: first accumulation region in its (bank x partition-group) —
# matmul start=True clears has_written for the whole bank x col-group,
# so only the first region per bank may use it (measured: a later
# start=True wiped the [512:600) accumulation state).
AB_WINS = [(0, 512, 0, True), (512, 600, 0, True),
           (600, 1024, 1, False), (1024, 1200, 1, True)]
FOLD_WINS = [(0, 512), (512, 1024), (1024, 1200)]

_CACHE = {}


def _build(scale_val: float):
    s_d = scale_val / D
    nc = bacc.Bacc("TRN2", debug=False, target_bir_lowering=False,
                   num_devices=NCORES)

    # DRAM I/O (host pre-arranged).  sup is padded to 128 partitions:
    # fewer engages only a subset of the 16 SDMA engines (measured 5/16
    # at 125 partitions -> 3x slower DMA).
    qt_dram = nc.dram_tensor("qt", [NG, 2, 128, DC, HC], BF16,
                             kind="ExternalInput")
    sup_dram = nc.dram_tensor("sup", [128, NPB, D], FP8,
                              kind="ExternalInput")
    oh_dram = nc.dram_tensor("oh", [PB * NS, PB * NW], FP8,
                             kind="ExternalInput")
    out_dram = nc.dram_tensor("out", [NG, 80, GC], BF16,
                              kind="ExternalOutput")

    with tile.TileContext(nc) as tc, ExitStack() as ctx:
        singles = ctx.enter_context(tc.tile_pool(name="singles", bufs=1))
        sq_pool = ctx.enter_context(tc.tile_pool(name="sq", bufs=3))
        cp_pool = ctx.enter_context(tc.tile_pool(name="cp", bufs=4))
        ps_pool = ctx.enter_context(
            tc.tile_pool(name="ps", bufs=2, space="PSUM"))

        # --- persistent SBUF tensors ---
        qt_sb = singles.tile([128, NG, 2, DC, HC], BF16)
        sup_sb = singles.tile([128, NPB, D], FP8)
        oh_sb = singles.tile([PB * NS, PB * NW], FP8)
        pt_sb = singles.tile([128, DC, PTC], BF16)      # protosT (0.4-scaled)
        logt_sb = singles.tile([80, NG, GC], BF16)
        ones_sb = singles.tile([128, 1], BF16)
        # K=33 fold operands: only partition rows 0 and 32 matter; rows
        # 1..31 are zero on the lhsT side.  All writes land at partition
        # base 0 or 32 (the BIR verifier rejects unaligned bases).
        fold_sb = singles.tile([33, NG, 80], BF16)      # r0=-BB/4, r32=-1
        aan_sb = singles.tile([33, NG, GC], BF16)       # r0=+1,    r32=AA
        sqp_sb = singles.tile([128, DC, PTC], BF16)     # protosT^2

        nc.vector.memset(ones_sb, 1.0)
        nc.vector.memset(fold_sb, 0.0)
        nc.vector.memset(fold_sb[32:33, :, :], -1.0)
        nc.vector.memset(aan_sb, 1.0)

        # --- input DMAs (sync HWDGE ring; drains in issue order) ---
        nc.sync.dma_start(out=oh_sb, in_=oh_dram.ap())
        nc.sync.dma_start(out=sup_sb, in_=sup_dram.ap())
        for g in range(NG):
            for h in range(2):
                nc.sync.dma_start(
                    out=qt_sb[:, g, h], in_=qt_dram.ap()[g, h])

        # --- protosT: per d-chunk, 13 blocks of 5 tasks (K=125).  The
        # psum tiles come from the same ring the AB groups use later.
        for c in range(DC):
            pt_ps = ps_pool.tile([128, GC], F32, tag="ab", name=f"ptps{c}")
            for b in range(NPB):
                t0 = PB * b
                ntask = min(PB, BPC - t0)
                kk = NS * ntask
                nc.tensor.matmul(
                    pt_ps[:, NW * t0:NW * (t0 + ntask)],
                    sup_sb[0:kk, b, 128 * c:128 * (c + 1)],
                    oh_sb[0:kk, 0:NW * ntask],
                    start=True, stop=True)
            nc.scalar.copy(out=pt_sb[:, c, :], in_=pt_ps[:, 0:PTC])

        # --- BB row: 0.25 * sum_d protosT^2 (DVE square, PE ones-reduce)
        nc.vector.tensor_mul(sqp_sb, pt_sb, pt_sb)
        bb_ps = ps_pool.tile([128, GC], F32, tag="ab", name="bbps")
        for c in range(DC):
            nc.tensor.matmul(bb_ps[0:1, 0:PTC], ones_sb, sqp_sb[:, c, :],
                             start=(c == 0), stop=(c == DC - 1))
        for g in range(NG):
            nc.vector.tensor_scalar(
                out=fold_sb[0:1, g, :], in0=bb_ps[0:1, 80 * g:80 * (g + 1)],
                scalar1=-0.25, scalar2=None, op0=mybir.AluOpType.mult)

        # --- per-group pipeline (issue order controls engine FIFOs) ---
        cps = {}

        def issue_sq(g, h):
            # squares of one qT half; ACT is 1x rate, DVE tensor_tensor
            # is 2x for bf16 -> split the two halves between them
            sq = sq_pool.tile([128, DC, HC], BF16, tag="sq")
            if h == 0:
                nc.scalar.activation(
                    out=sq, in_=qt_sb[:, g, h],
                    func=mybir.ActivationFunctionType.Square)
            else:
                nc.vector.tensor_mul(sq, qt_sb[:, g, h], qt_sb[:, g, h])
            cps[(g, h, "sq")] = sq

        def issue_adds(g, h):
            # DVE: one level of chunk-pair adds: 8 chunks -> 4 rows
            sq = cps.pop((g, h, "sq"))
            cp = cp_pool.tile([128, 4, HC], BF16, tag="cp")
            for c2 in range(4):
                nc.vector.tensor_add(cp[:, c2, :], sq[:, 2 * c2, :],
                                     sq[:, 2 * c2 + 1, :])
            cps[(g, h)] = cp

        def issue_ab(g, abt, h):
            # AB^T matmuls for the windows fed by qT half h
            for c in range(DC):
                lhs = pt_sb[:, c, 80 * g:80 * (g + 1)]
                for (a, b_, hh, first) in AB_WINS:
                    if hh != h:
                        continue
                    off = HC * h
                    nc.tensor.matmul(
                        abt[0:80, a:b_], lhs,
                        qt_sb[:, g, h, c, a - off:b_ - off],
                        start=(c == 0 and first), stop=False)

        def issue_minis(g, abt, h):
            # AA partition-reduce into spare psum row 96 (32-aligned)
            cp = cps.pop((g, h))
            off = HC * h
            for (a, b_, hh, first) in AB_WINS:
                if hh != h:
                    continue
                for c2 in range(4):
                    nc.tensor.matmul(
                        abt[96:97, a:b_], ones_sb,
                        cp[:, c2, a - off:b_ - off],
                        start=(c2 == 0 and first), stop=(c2 == 3),
                        tile_position=(0, 96))

        def issue_aan(g, abt, h):
            # psum AA row -> bf16 fold operand (row 32 of aan_sb).
            # DVE only: ACT mis-handles partition-base-shifting copies.
            cs = slice(HC * h, HC * (h + 1))
            nc.vector.tensor_copy(aan_sb[32:33, g, cs], abt[96:97, cs])

        def issue_fold(g, abt):
            # K=33 fold: out += (-BB/4)[row]*1 + (-1)*AA[col]
            for (a, b_) in FOLD_WINS:
                nc.tensor.matmul(
                    abt[0:80, a:b_], fold_sb[:, g, :],
                    aan_sb[:, g, a:b_], start=False, stop=True)

        def issue_out(g, abt):
            # ACT: psum -> logitsT bf16 with scale/d, then store
            nc.scalar.activation(
                out=logt_sb[:, g, :], in_=abt[0:80, :],
                func=mybir.ActivationFunctionType.Copy, scale=s_d)
            nc.scalar.dma_start(out=out_dram.ap()[g], in_=logt_sb[:, g, :])

        issue_sq(0, 0)
        issue_sq(0, 1)
        issue_adds(0, 0)
        issue_adds(0, 1)
        abts = {}
        for g in range(NG + 1):
            if g >= 1:
                issue_out(g - 1, abts.pop(g - 1))
            if g >= NG:
                break
            abt = ps_pool.tile([128, GC], F32, tag="ab", name=f"abt{g}")
            abts[g] = abt
            issue_ab(g, abt, 0)
            issue_minis(g, abt, 0)
            issue_aan(g, abt, 0)
            if g + 1 < NG:
                issue_sq(g + 1, 0)
            issue_ab(g, abt, 1)
            issue_minis(g, abt, 1)
            issue_aan(g, abt, 1)
            if g + 1 < NG:
                issue_sq(g + 1, 1)
                issue_adds(g + 1, 0)
                issue_adds(g + 1, 1)
            issue_fold(g, abt)

    nc.compile()
    return nc


def _host_prep(query, support, labels, n_way, n_shot):
    """Per-core input maps: layout transforms + dtype casts only."""
    q = np.asarray(query, dtype=np.float32)
    sup = np.asarray(support, dtype=np.float32)
    lab = np.asarray(labels).astype(np.int64)

    # sort support per task by label so class w occupies slots 5w..5w+5
    order = np.argsort(lab, axis=1, kind="stable")          # (B, 25)
    counts = (lab[:, :, None] == np.arange(n_way)[None, None, :]).sum(1)
    assert np.all(counts == n_shot), "kernel assumes exact n_shot per class"
    sup_sorted = np.take_along_axis(sup, order[:, :, None], axis=1)

    # 2/n_shot pre-scale makes the AB matmul psum equal 2*q@protos.T
    sup_f8 = (sup_sorted * (2.0 / n_shot)).astype(ml_dtypes.float8_e4m3)
    q_bf = q.astype(ml_dtypes.bfloat16)

    # block-diagonal 0/1 one-hot shared by every task (labels sorted)
    oh = np.zeros((PB * NS, PB * NW), dtype=ml_dtypes.float8_e4m3)
    for j in range(PB):
        for w in range(NW):
            oh[NS * j + NSHOT * w:NS * j + NSHOT * (w + 1), NW * j + w] = 1.0

    in_maps = []
    for cidx in range(NCORES):
        t0 = BPC * cidx
        # qT: (64, 75, 1024) -> (g, h, dl, c, 8*75)
        qc = q_bf[t0:t0 + BPC].reshape(NG, 2, 8, NQ, DC, 128)
        qc = np.ascontiguousarray(qc.transpose(0, 1, 5, 4, 2, 3)).reshape(
            NG, 2, 128, DC, HC)
        # support: 13 blocks of 5 tasks, (128-padded partitions, block, d)
        sc = np.zeros((128, NPB, D), dtype=ml_dtypes.float8_e4m3)
        st = sup_f8[t0:t0 + BPC]                            # (64, 25, 1024)
        full = st[:(NPB - 1) * PB].reshape(NPB - 1, PB * NS, D)
        sc[:PB * NS, :NPB - 1, :] = full.transpose(1, 0, 2)
        rem = st[(NPB - 1) * PB:]                           # last 4 tasks
        sc[:rem.shape[0] * NS, NPB - 1, :] = rem.reshape(-1, D)
        in_maps.append({"qt": qc, "sup": np.ascontiguousarray(sc), "oh": oh})
    return in_maps


TRACE = False
last_exec_time_ns = None


def kernel(**inputs):
    global last_exec_time_ns
    query = inputs["query"]
    support = inputs["support"]
    labels = inputs["support_labels"]
    n_way = int(np.asarray(inputs.get("n_way", NW)))
    n_shot = int(np.asarray(inputs.get("n_shot", NSHOT)))
    scale = float(np.asarray(inputs["scale"]).reshape(-1)[0])
    assert n_way == NW and n_shot == NSHOT

    key = scale
    if key not in _CACHE:
        _CACHE[key] = _build(scale)
    nc = _CACHE[key]

    in_maps = _host_prep(query, support, labels, n_way, n_shot)
    res = bass_utils.run_bass_kernel_spmd(
        nc, in_maps, core_ids=list(range(NCORES)), trace=TRACE)
    last_exec_time_ns = res.exec_time_ns

    # host-side output untangle: (g, 80, 1200) -> diag blocks -> (64, 75, 5)
    idx = np.arange(TG)
    outs = []
    for cidx in range(NCORES):
        lt = np.asarray(res.results[cidx]["out"], dtype=np.float32)
        lt = lt.reshape(NG, TG, NW, TG, NQ).transpose(0, 1, 3, 2, 4)
        diag = lt[:, idx, idx]                    # (NG, TG, NW, NQ)
        outs.append(diag.transpose(0, 1, 3, 2).reshape(BPC, NQ, NW))
    return np.concatenate(outs, axis=0).astype(np.float32)


# revision 19
# speedup vs baseline: 1.0071x; 1.0071x over previous
"""Trainium2 Bass kernel for a prototypical-network classification head.

Math (per task b):
    protos  = one_hot(labels).T @ support / n_shot          # (5, 1024)
    logits  = scale/d * (2*q@protos.T - |q|^2 - |p|^2)       # (75, 5)

Sharding: data-parallel over the 512 tasks across 8 NeuronCores (64 each).

v5 dataflow (zero on-device transposes):
  - Host pre-transposes query to qT (d on partitions, bf16) so the PE
    never transposes anything.  Host label-sorts support, pre-scales it
    by 2/n_shot and casts fp8e4m3 (support only touches the logits
    through class-mean inner products; measured end-to-end rel err
    7.2e-3 vs the 2e-2 gate).
  - protosT built by PE matmuls: stationary = natural-layout support
    chunk (s on partitions, 128 d columns), moving = a tiny
    block-diagonal 0/1 one-hot shared by all tasks.
  - AB^T accumulates in a 3-bank (128, 1200) psum tile per 16-task
    group over 8 d-chunks: stationary = protosT slice (128, 80).
  - AA = |q|^2 per query: ACT/DVE split squares of qT, DVE adds chunk
    pairs, a ones-vector PE matmul reduces partitions into spare psum
    row 96 of the same AB psum tile.  BB likewise from protosT.
  - One K=33 "fold" matmul per psum bank window adds -AA[col] - BB[row]
    (rows 0/32 of the operands are the written lanes; partition bases
    stay 32-aligned for the BIR verifier).  The psum->SBUF copy applies
    scale/d and casts bf16.  Transposed logits ship to DRAM; the host
    extracts each task's (5, 75) diagonal block and transposes it
    (layout-only numpy work).
"""

import numpy as np
from contextlib import ExitStack

import ml_dtypes
import concourse.bass as bass
import concourse.bacc as bacc
import concourse.tile as tile
from concourse import mybir
from concourse import bass_utils

F32 = mybir.dt.float32
BF16 = mybir.dt.bfloat16
FP8 = mybir.dt.float8e4

# Problem shape (hardcoded per the task spec).
B, NQ, NS, D = 512, 75, 25, 1024
NW, NSHOT = 5, 5
NCORES = 8
BPC = B // NCORES          # 64 tasks per core
DC = D // 128              # 8 contraction chunks

TG = 16                    # tasks per AB group
NG = BPC // TG             # 4 groups
GC = TG * NQ               # 1200 q-columns per group
HC = GC // 2               # 600 q-columns per DMA half
PB = 5                     # tasks per protos block (K = 125)
NPB = (BPC + PB - 1) // PB  # 13 blocks (last has 4 tasks)
PTC = BPC * NW             # 320 protosT columns

# psum-bank-aligned column windows of the (128, 1200) group tile, split
# at the DMA-half boundary (qT halves live in separate SBUF regions).
# 4th field: first accumulation region in its (bank x partition-group) —
# matmul start=True clears has_written for the whole bank x col-group,
# so only the first region per bank may use it (measured: a later
# start=True wiped the [512:600) accumulation state).
AB_WINS = [(0, 512, 0, True), (512, 600, 0, True),
           (600, 1024, 1, False), (1024, 1200, 1, True)]
FOLD_WINS = [(0, 512), (512, 1024), (1024, 1200)]

_CACHE = {}


def _build(scale_val: float):
    s_d = scale_val / D
    nc = bacc.Bacc("TRN2", debug=False, target_bir_lowering=False,
                   num_devices=NCORES)

    # DRAM I/O (host pre-arranged).  sup is padded to 128 partitions:
    # fewer engages only a subset of the 16 SDMA engines (measured 5/16
    # at 125 partitions -> 3x slower DMA).
    qt_dram = nc.dram_tensor("qt", [NG, 2, 128, DC, HC], BF16,
                             kind="ExternalInput")
    sup_dram = nc.dram_tensor("sup", [128, NPB, D], FP8,
                              kind="ExternalInput")
    oh_dram = nc.dram_tensor("oh", [PB * NS, PB * NW], FP8,
                             kind="ExternalInput")
    out_dram = nc.dram_tensor("out", [NG, 80, GC], BF16,
                              kind="ExternalOutput")

    with tile.TileContext(nc) as tc, ExitStack() as ctx:
        singles = ctx.enter_context(tc.tile_pool(name="singles", bufs=1))
        sq_pool = ctx.enter_context(tc.tile_pool(name="sq", bufs=3))
        cp_pool = ctx.enter_context(tc.tile_pool(name="cp", bufs=4))
        ps_pool = ctx.enter_context(
            tc.tile_pool(name="ps", bufs=2, space="PSUM"))

        # --- persistent SBUF tensors ---
        qt_sb = singles.tile([128, NG, 2, DC, HC], BF16)
        sup_sb = singles.tile([128, NPB, D], FP8)
        oh_sb = singles.tile([PB * NS, PB * NW], FP8)
        # padded to 128 cols per group so AB stationaries are FWL-eligible
        pt_sb = singles.tile([128, DC, NG, 128], BF16)  # protosT (0.4-scaled)
        logt_sb = singles.tile([80, NG, GC], BF16)
        ones_sb = singles.tile([128, 1], BF16)
        # K=33 fold operands: only partition rows 0 and 32 matter; rows
        # 1..31 are zero on the lhsT side.  All writes land at partition
        # base 0 or 32 (the BIR verifier rejects unaligned bases).
        fold_sb = singles.tile([33, NG, 80], BF16)      # r0=-BB/4, r32=-1
        aan_sb = singles.tile([33, NG, GC], BF16)       # r0=+1,    r32=AA
        sqp_sb = singles.tile([128, DC, PTC], BF16)     # protosT^2

        nc.vector.memset(ones_sb, 1.0)
        nc.vector.memset(pt_sb, 0.0)
        nc.vector.memset(fold_sb, 0.0)
        nc.vector.memset(fold_sb[32:33, :, :], -1.0)
        nc.vector.memset(aan_sb, 1.0)

        # --- input DMAs (sync HWDGE ring; drains in issue order) ---
        nc.sync.dma_start(out=oh_sb, in_=oh_dram.ap())
        nc.sync.dma_start(out=sup_sb, in_=sup_dram.ap())
        for g in range(NG):
            for h in range(2):
                nc.sync.dma_start(
                    out=qt_sb[:, g, h], in_=qt_dram.ap()[g, h])

        # --- protosT: per d-chunk, 13 blocks of 5 tasks (K=125).  The
        # psum tiles come from the same ring the AB groups use later.
        for c in range(DC):
            pt_ps = ps_pool.tile([128, GC], F32, tag="ab", name=f"ptps{c}")
            for b in range(NPB):
                t0 = PB * b
                ntask = min(PB, BPC - t0)
                kk = NS * ntask
                nc.tensor.matmul(
                    pt_ps[:, NW * t0:NW * (t0 + ntask)],
                    sup_sb[0:kk, b, 128 * c:128 * (c + 1)],
                    oh_sb[0:kk, 0:NW * ntask],
                    start=True, stop=True)
            nc.scalar.copy(
                out=pt_sb[:, c, :, 0:80],
                in_=pt_ps[:, 0:PTC].rearrange("p (g w) -> p g w", g=NG))

        # --- BB row: 0.25 * sum_d protosT^2 (DVE square, PE ones-reduce).
        # Issued mid-pipeline (only needed by fold(g0)) so the DVE/PE
        # queues aren't head-of-line blocked waiting for protosT.
        def issue_bb(bb_ps):
            sqpv = sqp_sb.rearrange("p c (g w) -> p c g w", g=NG)
            nc.vector.tensor_mul(sqpv, pt_sb[:, :, :, 0:80],
                                 pt_sb[:, :, :, 0:80])
            for c in range(DC):
                nc.tensor.matmul(bb_ps[0:1, 0:PTC], ones_sb, sqp_sb[:, c, :],
                                 start=(c == 0), stop=(c == DC - 1))
            for g in range(NG):
                nc.vector.tensor_scalar(
                    out=fold_sb[0:1, g, :],
                    in0=bb_ps[0:1, 80 * g:80 * (g + 1)],
                    scalar1=-0.25, scalar2=None, op0=mybir.AluOpType.mult)

        # --- per-group pipeline (issue order controls engine FIFOs) ---
        cps = {}

        def issue_sq(g, h):
            # squares of one qT half; ACT is 1x rate, DVE tensor_tensor
            # is 2x for bf16 -> split the two halves between them
            sq = sq_pool.tile([128, DC, HC], BF16, tag="sq")
            if h == 0:
                nc.scalar.activation(
                    out=sq, in_=qt_sb[:, g, h],
                    func=mybir.ActivationFunctionType.Square)
            else:
                nc.vector.tensor_mul(sq, qt_sb[:, g, h], qt_sb[:, g, h])
            cps[(g, h, "sq")] = sq

        def issue_adds(g, h):
            # DVE: one level of chunk-pair adds: 8 chunks -> 4 rows
            sq = cps.pop((g, h, "sq"))
            cp = cp_pool.tile([128, 4, HC], BF16, tag="cp")
            for c2 in range(4):
                nc.vector.tensor_add(cp[:, c2, :], sq[:, 2 * c2, :],
                                     sq[:, 2 * c2 + 1, :])
            cps[(g, h)] = cp

        def issue_ab(g, abt, h):
            # AB^T matmuls for the windows fed by qT half h
            for c in range(DC):
                lhs = pt_sb[:, c, g, :]
                for (a, b_, hh, first) in AB_WINS:
                    if hh != h:
                        continue
                    off = HC * h
                    nc.tensor.matmul(
                        abt[:, a:b_], lhs,
                        qt_sb[:, g, h, c, a - off:b_ - off],
                        start=(c == 0 and first), stop=False)

        def issue_minis(g, abt, h):
            # AA partition-reduce into spare psum row 96 (32-aligned)
            cp = cps.pop((g, h))
            off = HC * h
            for (a, b_, hh, first) in AB_WINS:
                if hh != h:
                    continue
                for c2 in range(4):
                    # start=False: the (now M=128) AB matmuls already
                    # wrote zeros to row 96 (zero stationary pad cols)
                    nc.tensor.matmul(
                        abt[96:97, a:b_], ones_sb,
                        cp[:, c2, a - off:b_ - off],
                        start=False, stop=(c2 == 3),
                        tile_position=(0, 96))

        def issue_aan(g, abt, h):
            # psum AA row -> bf16 fold operand (row 32 of aan_sb)
            cs = slice(HC * h, HC * (h + 1))
            if h == 0:
                nc.scalar.copy(out=aan_sb[32:33, g, cs], in_=abt[96:97, cs])
            else:
                nc.vector.tensor_copy(aan_sb[32:33, g, cs], abt[96:97, cs])

        def issue_fold(g, abt):
            # K=33 fold: out += (-BB/4)[row]*1 + (-1)*AA[col]
            for (a, b_) in FOLD_WINS:
                nc.tensor.matmul(
                    abt[0:80, a:b_], fold_sb[:, g, :],
                    aan_sb[:, g, a:b_], start=False, stop=True)

        def issue_out(g, abt):
            # ACT: psum -> logitsT bf16 with scale/d, then store
            nc.scalar.activation(
                out=logt_sb[:, g, :], in_=abt[0:80, :],
                func=mybir.ActivationFunctionType.Copy, scale=s_d)
            nc.scalar.dma_start(out=out_dram.ap()[g], in_=logt_sb[:, g, :])

        # reserve the BB psum ring slot before abt0 so the abt ring
        # keeps two groups in flight (slot order is allocation order)
        bb_ps = ps_pool.tile([128, GC], F32, tag="ab", name="bbps")
        issue_sq(0, 0)
        issue_sq(0, 1)
        issue_adds(0, 0)
        issue_adds(0, 1)
        abts = {}
        for g in range(NG + 1):
            if g >= 1:
                issue_out(g - 1, abts.pop(g - 1))
            if g >= NG:
                break
            abt = ps_pool.tile([128, GC], F32, tag="ab", name=f"abt{g}")
            abts[g] = abt
            issue_ab(g, abt, 0)
            issue_minis(g, abt, 0)
            issue_aan(g, abt, 0)
            if g == 0:
                issue_bb(bb_ps)
            if g + 1 < NG:
                issue_sq(g + 1, 0)
            issue_ab(g, abt, 1)
            issue_minis(g, abt, 1)
            issue_aan(g, abt, 1)
            if g + 1 < NG:
                issue_sq(g + 1, 1)
                issue_adds(g + 1, 0)
                issue_adds(g + 1, 1)
            issue_fold(g, abt)

    nc.compile()
    return nc


def _host_prep(query, support, labels, n_way, n_shot):
    """Per-core input maps: layout transforms + dtype casts only."""
    q = np.asarray(query, dtype=np.float32)
    sup = np.asarray(support, dtype=np.float32)
    lab = np.asarray(labels).astype(np.int64)

    # sort support per task by label so class w occupies slots 5w..5w+5
    order = np.argsort(lab, axis=1, kind="stable")          # (B, 25)
    counts = (lab[:, :, None] == np.arange(n_way)[None, None, :]).sum(1)
    assert np.all(counts == n_shot), "kernel assumes exact n_shot per class"
    sup_sorted = np.take_along_axis(sup, order[:, :, None], axis=1)

    # 2/n_shot pre-scale makes the AB matmul psum equal 2*q@protos.T
    sup_f8 = (sup_sorted * (2.0 / n_shot)).astype(ml_dtypes.float8_e4m3)
    q_bf = q.astype(ml_dtypes.bfloat16)

    # block-diagonal 0/1 one-hot shared by every task (labels sorted)
    oh = np.zeros((PB * NS, PB * NW), dtype=ml_dtypes.float8_e4m3)
    for j in range(PB):
        for w in range(NW):
            oh[NS * j + NSHOT * w:NS * j + NSHOT * (w + 1), NW * j + w] = 1.0

    in_maps = []
    for cidx in range(NCORES):
        t0 = BPC * cidx
        # qT: (64, 75, 1024) -> (g, h, dl, c, 8*75)
        qc = q_bf[t0:t0 + BPC].reshape(NG, 2, 8, NQ, DC, 128)
        qc = np.ascontiguousarray(qc.transpose(0, 1, 5, 4, 2, 3)).reshape(
            NG, 2, 128, DC, HC)
        # support: 13 blocks of 5 tasks, (128-padded partitions, block, d)
        sc = np.zeros((128, NPB, D), dtype=ml_dtypes.float8_e4m3)
        st = sup_f8[t0:t0 + BPC]                            # (64, 25, 1024)
        full = st[:(NPB - 1) * PB].reshape(NPB - 1, PB * NS, D)
        sc[:PB * NS, :NPB - 1, :] = full.transpose(1, 0, 2)
        rem = st[(NPB - 1) * PB:]                           # last 4 tasks
        sc[:rem.shape[0] * NS, NPB - 1, :] = rem.reshape(-1, D)
        in_maps.append({"qt": qc, "sup": np.ascontiguousarray(sc), "oh": oh})
    return in_maps


TRACE = False
last_exec_time_ns = None


def kernel(**inputs):
    global last_exec_time_ns
    query = inputs["query"]
    support = inputs["support"]
    labels = inputs["support_labels"]
    n_way = int(np.asarray(inputs.get("n_way", NW)))
    n_shot = int(np.asarray(inputs.get("n_shot", NSHOT)))
    scale = float(np.asarray(inputs["scale"]).reshape(-1)[0])
    assert n_way == NW and n_shot == NSHOT

    key = scale
    if key not in _CACHE:
        _CACHE[key] = _build(scale)
    nc = _CACHE[key]

    in_maps = _host_prep(query, support, labels, n_way, n_shot)
    res = bass_utils.run_bass_kernel_spmd(
        nc, in_maps, core_ids=list(range(NCORES)), trace=TRACE)
    last_exec_time_ns = res.exec_time_ns

    # host-side output untangle: (g, 80, 1200) -> diag blocks -> (64, 75, 5)
    idx = np.arange(TG)
    outs = []
    for cidx in range(NCORES):
        lt = np.asarray(res.results[cidx]["out"], dtype=np.float32)
        lt = lt.reshape(NG, TG, NW, TG, NQ).transpose(0, 1, 3, 2, 4)
        diag = lt[:, idx, idx]                    # (NG, TG, NW, NQ)
        outs.append(diag.transpose(0, 1, 3, 2).reshape(BPC, NQ, NW))
    return np.concatenate(outs, axis=0).astype(np.float32)
